# revision 35
# baseline (speedup 1.0000x reference)
"""Single-head MHA (QKV proj + softmax attention) on 8 Trainium2 cores.

Problem: x[8, 4096, 256] f32; per-batch attention with per-head emb 256.
Sharding: data-parallel — one batch element per NeuronCore (8 cores).

Per-core algorithm (S=4096, E=256, P=128 partitions):
  - transpose x -> xT[d, s] (PE transpose, bf16)
  - QT[e, s] = Wq @ xT (+bq), KT likewise, V[s, e] = xT.T @ WvT  (bf16)
  - per q-block of 512 columns, loop k-tiles of 128 rows:
      S^T[k, q] = KT_slice.T @ QT  (2 matmuls, fp32 PSUM)
      E[k, q]   = exp(S^T / 16)    (ScalarE, scale fused, bf16 out)
      outT[e,q]+= V_slice.T @ E    (2 matmuls, fp32 PSUM accumulation)
      denom    += E                (VectorE, fp32 accumulator)
    softmax denominators: PE-transpose denom blocks + free-dim reduce,
    reciprocal; out blocks: PE-transpose outT, scale by recip (per-partition
    broadcast), + bv (softmax rows sum to 1, so attn @ (V + bv) = attn@V + bv).

No running-max subtraction: scores/16 ~ N(0,1); max over 16.8M draws < ~6,
exp stays well inside fp32/bf16 range.
"""

from contextlib import ExitStack

import numpy as np

import concourse.bass as bass
import concourse.tile as tile
from concourse import bacc
from concourse import mybir
from concourse import bass_utils
from concourse.masks import make_identity

P = 128          # partitions
EMB = 256        # head dim
S = 4096         # sequence length
B = 8            # batch == number of cores
QB = 1024        # q-block (free dim of S^T / E tiles; matmuls split into 512s)
MMN = 512        # max matmul free dim (one PSUM bank of fp32)

F32 = mybir.dt.float32
BF16 = mybir.dt.bfloat16
AF = mybir.ActivationFunctionType


def _build(nc: bass.Bass, s_len: int = S) -> None:
    """Emit the per-core program into `nc` (SPMD: same program all cores)."""
    x = nc.dram_tensor("x", (s_len, EMB), F32, kind="ExternalInput").ap()
    Wq = nc.dram_tensor("Wq", (EMB, EMB), F32, kind="ExternalInput").ap()
    bq = nc.dram_tensor("bq", (EMB,), F32, kind="ExternalInput").ap()
    Wk = nc.dram_tensor("Wk", (EMB, EMB), F32, kind="ExternalInput").ap()
    bk = nc.dram_tensor("bk", (EMB,), F32, kind="ExternalInput").ap()
    Wv = nc.dram_tensor("Wv", (EMB, EMB), F32, kind="ExternalInput").ap()
    bv = nc.dram_tensor("bv", (EMB,), F32, kind="ExternalInput").ap()
    out = nc.dram_tensor("out", (s_len, EMB), F32, kind="ExternalOutput").ap()

    n_st = s_len // P      # 128-row tiles of the sequence
    n_qb = s_len // QB     # q-blocks
    n_kt = s_len // P      # k-tiles
    n_qt = QB // P         # 128-row q-tiles per q-block
    scale = float(EMB) ** -0.5

    with tile.TileContext(nc) as tc, ExitStack() as ctx:
        consts = ctx.enter_context(tc.tile_pool(name="consts", bufs=1))
        persist = ctx.enter_context(tc.tile_pool(name="persist", bufs=1))
        stage = ctx.enter_context(tc.tile_pool(name="stage", bufs=3))
        work = ctx.enter_context(tc.tile_pool(name="work", bufs=5))
        outp = ctx.enter_context(tc.tile_pool(name="outp", bufs=4))
        # PSUM budget is exactly 8 banks: "mm" 2 slots x [128,1024]f32 (2
        # banks each) + "po" 2 slots x [128,1024]f32. Transpose outputs and
        # projection tiles share the "mm" slots.
        ps_mm = ctx.enter_context(tc.tile_pool(name="ps_mm", bufs=2, space="PSUM"))
        ps_acc = ctx.enter_context(tc.tile_pool(name="ps_acc", bufs=1, space="PSUM"))
        ps_tp = ps_mm

        # identity for PE transposes (all transposes run in f32; the cast to
        # bf16 happens on the PSUM->SBUF copy)
        idf = consts.tile([P, P], F32)
        make_identity(nc, idf)
        ones_f = consts.tile([P, 1], F32)
        nc.vector.memset(ones_f, 1.0)
        idb = consts.tile([P, P], BF16)
        nc.vector.tensor_copy(idb, idf)

        # biases: bq/bk as per-partition columns (e on partitions),
        # bv broadcast across partitions (added at the very end).
        bq_sb = consts.tile([P, 2], F32)
        nc.gpsimd.dma_start(bq_sb, bq.rearrange("(t p) -> p t", p=P))
        bk_sb = consts.tile([P, 2], F32)
        nc.gpsimd.dma_start(bk_sb, bk.rearrange("(t p) -> p t", p=P))
        bv_bc = consts.tile([P, EMB], F32)
        nc.gpsimd.dma_start(
            bv_bc,
            bass.AP(tensor=bv.tensor, offset=bv.offset, ap=[[0, P], list(bv.ap[0])]),
        )

        # ---- weights: load W[e,d], PE-transpose -> WT[d,e] (cast on copy) ----
        WT = {}
        for wname, wap in (("q", Wq), ("k", Wk), ("v", Wv)):
            wt0 = persist.tile([P, EMB], BF16, name=f"wt_{wname}_0")
            wt1 = persist.tile([P, EMB], BF16, name=f"wt_{wname}_1")
            WT[wname] = (wt0, wt1)
            for et in range(2):
                wst = stage.tile([P, EMB], F32, tag="wst")
                nc.sync.dma_start(wst, wap[et * P:(et + 1) * P, :])
                wbf = stage.tile([P, EMB], BF16, tag="wbf")
                nc.vector.tensor_copy(wbf, wst)
                for dc in range(2):
                    tp = ps_tp.tile([P, P], BF16, tag="mm")
                    nc.tensor.transpose(tp, wbf[:, dc * P:(dc + 1) * P], idb)
                    nc.scalar.copy(WT[wname][dc][:, et * P:(et + 1) * P], tp)

        # ---- x: load, PE-transpose -> xT[dc][d, s] (cast on copy) ----
        xT = [persist.tile([P, s_len], BF16, name=f"xT{dc}") for dc in range(2)]
        for st_i in range(n_st):
            xst = stage.tile([P, EMB], F32, tag="xst")
            nc.sync.dma_start(xst, x[st_i * P:(st_i + 1) * P, :])
            xbf = stage.tile([P, EMB], BF16, tag="xbf")
            nc.vector.tensor_copy(xbf, xst)
            for dc in range(2):
                tp = ps_tp.tile([P, P], BF16, tag="mm")
                nc.tensor.transpose(tp, xbf[:, dc * P:(dc + 1) * P], idb)
                nc.scalar.copy(xT[dc][:, st_i * P:(st_i + 1) * P], tp)

        # ---- projections: QT/KT[e, s] (e on partitions), V[s, e] ----
        QT = [persist.tile([P, s_len], BF16, name=f"QT{t}") for t in range(2)]
        KT = [persist.tile([P, s_len], BF16, name=f"KT{t}") for t in range(2)]
        Vb = persist.tile([P, n_st, EMB], BF16, name="Vb")
        # K first, then V, then Q: the attention loop needs all of K/V but
        # only the first q-block of Q, so this order unblocks it earliest.
        for t in range(2):
            for sb in range(s_len // MMN):
                ssl = slice(sb * MMN, (sb + 1) * MMN)
                kps = ps_mm.tile([P, MMN], F32, tag="mm")
                nc.tensor.matmul(kps, WT["k"][0][:, t * P:(t + 1) * P],
                                 xT[0][:, ssl], start=True, stop=False)
                nc.tensor.matmul(kps, WT["k"][1][:, t * P:(t + 1) * P],
                                 xT[1][:, ssl], start=False, stop=True)
                nc.vector.tensor_scalar_add(KT[t][:, ssl], kps,
                                            bk_sb[:, t:t + 1])
        for st_i in range(n_st):
            vps = ps_mm.tile([P, EMB], F32, tag="mm")
            nc.tensor.matmul(vps, xT[0][:, st_i * P:(st_i + 1) * P], WT["v"][0],
                             start=True, stop=False)
            nc.tensor.matmul(vps, xT[1][:, st_i * P:(st_i + 1) * P], WT["v"][1],
                             start=False, stop=True)
            nc.vector.tensor_copy(Vb[:, st_i, :], vps)
        for sb in range(s_len // MMN):
            ssl = slice(sb * MMN, (sb + 1) * MMN)
            for t in range(2):
                qps = ps_mm.tile([P, MMN], F32, tag="mm")
                nc.tensor.matmul(qps, WT["q"][0][:, t * P:(t + 1) * P],
                                 xT[0][:, ssl], start=True, stop=False)
                nc.tensor.matmul(qps, WT["q"][1][:, t * P:(t + 1) * P],
                                 xT[1][:, ssl], start=False, stop=True)
                nc.vector.tensor_scalar_add(QT[t][:, ssl], qps,
                                            bq_sb[:, t:t + 1])

        # ---- attention ----
        ebf_chunks = n_qt  # 128-wide q-chunks of the exp tile
        for qb_i in range(n_qb):
            q0b = qb_i * QB
            # out accumulator in [q, e] layout: one PSUM tensor, 4 banks.
            # PV uses the exp tile E^T[k, q] as the *stationary* operand
            # (128-column q-chunks) and V[k, e] as the moving operand, so the
            # attention output lands directly in [q, e] — no transposes.
            out_ps = ps_acc.tile([P, n_qt, EMB], F32, tag="po",
                                 name=f"out_ps_{qb_i}")
            # two interleaved softmax-denominator accumulators, one summed on
            # the vector engine and one on gpsimd, so neither chain bottlenecks
            dacc = [work.tile([P, QB], F32, tag=f"dacc{i}", name=f"dacc{i}_{qb_i}")
                    for i in range(2)]
            e_prev = None
            for kt_i in range(n_kt):
                ksl = slice(kt_i * P, (kt_i + 1) * P)
                stp = ps_mm.tile([P, QB], F32, tag="mm")
                for t in range(2):
                    # lhsT (KT slice) is reused by both q-halves: one
                    # LDWEIGHTS serves two matmuls.
                    for qh in range(2):
                        hs = slice(qh * MMN, (qh + 1) * MMN)
                        nc.tensor.matmul(stp[:, hs], KT[t][:, ksl],
                                         QT[t][:, q0b + qh * MMN:
                                                q0b + (qh + 1) * MMN],
                                         start=(t == 0), stop=(t == 1))
                if e_prev is not None:
                    # PV for the previous k-tile: keeps PE busy while ACT
                    # computes this tile's exp (software pipelining).
                    kp = kt_i - 1
                    for j in range(n_qt):
                        # PSUM accumulation groups are bank-granular: two
                        # adjacent j-slices share a 2KB bank, so the group
                        # opens on the even slice and closes on the odd one.
                        nc.tensor.matmul(out_ps[:, j, :],
                                         e_prev[:, j * P:(j + 1) * P],
                                         Vb[:, kp, :],
                                         start=(kp == 0 and j % 2 == 0),
                                         stop=False)
                ebf = work.tile([P, QB], BF16, tag="E")
                nc.scalar.activation(ebf, stp, AF.Exp, scale=scale)
                idx = kt_i % 2
                eng = nc.vector
                if kt_i < 2:
                    eng.tensor_copy(dacc[idx], ebf)
                else:
                    eng.tensor_add(dacc[idx], dacc[idx], ebf)
                e_prev = ebf
            for j in range(n_qt):
                nc.tensor.matmul(out_ps[:, j, :],
                                 e_prev[:, j * P:(j + 1) * P],
                                 Vb[:, n_kt - 1, :],
                                 start=False, stop=(j % 2 == 1))

            dall = work.tile([P, QB], F32, tag="dall")
            nc.vector.tensor_add(dall, dacc[0], dacc[1])

            # softmax denominators via tiny N=1 matmuls: dn[:, j] =
            # dall_chunk_j.T @ ones -> [q, 1] columns, accumulated in a single
            # PSUM bank (group opens at j=0, closes at j=n_qt-1).
            dn_ps = ps_mm.tile([P, n_qt], F32, tag="mm", name=f"dn_{qb_i}")
            for j in range(n_qt):
                nc.tensor.matmul(dn_ps[:, j:j + 1],
                                 dall[:, j * P:(j + 1) * P], ones_f,
                                 start=(j == 0), stop=(j == n_qt - 1))
            recip = work.tile([P, n_qt], F32, tag="recip")
            nc.vector.reciprocal(recip, dn_ps)

            # finalize: out = out_ps*recip + bv, fused in one DVE op per tile
            for j in range(n_qt):
                res = outp.tile([P, EMB], F32, tag="res")
                nc.vector.scalar_tensor_tensor(
                    res, out_ps[:, j, :], recip[:, j:j + 1], bv_bc,
                    op0=mybir.AluOpType.mult, op1=mybir.AluOpType.add)
                q0 = q0b + j * P
                nc.sync.dma_start(out[q0:q0 + P, :], res)


def _make_nc(s_len: int = S) -> bass.Bass:
    # Bacc (not raw Bass): its compile() splits multi-sem waits and moves
    # matmul waits onto ldweights — HW allows at most one wait per inst.
    nc = bacc.Bacc("TRN2", target_bir_lowering=False, debug=False)
    _build(nc, s_len)
    nc.compile()
    return nc


def _prep(inputs: dict) -> dict:
    arrs = {k: np.ascontiguousarray(np.asarray(v, dtype=np.float32))
            for k, v in inputs.items()}
    assert arrs["x"].shape == (B, S, EMB), arrs["x"].shape
    return arrs


def run(inputs: dict):
    """Run on 8 NeuronCores. Returns (out[B,S,E] f32, BassKernelResults)."""
    arrs = _prep(inputs)
    nc = _make_nc(S)
    shared = {k: arrs[k] for k in ("Wq", "bq", "Wk", "bk", "Wv", "bv")}
    in_maps = [dict(shared, x=arrs["x"][i]) for i in range(B)]
    res = bass_utils.run_bass_kernel_spmd(nc, in_maps, core_ids=list(range(B)))
    out = np.stack([r["out"] for r in res.results], axis=0).astype(np.float32)
    return out, res


def kernel(**inputs) -> np.ndarray:
    out, _ = run(inputs)
    return out


def bench(inputs: dict, iters: int = 5, chain: int = 1):
    """Compile once, then time repeated executions with device-resident
    inputs (mirrors bass2jax.run_bass_via_pjrt's multi-core path).

    `chain` > 1 executes the NEFF that many times inside one XLA program
    (each call's outputs feed the next call's donated output buffers, which
    serializes them) so per-iteration device time can be extracted as a
    slope, amortizing the axon dispatch overhead.

    Returns (out[B,S,E] f32, list of per-call wall times in seconds).
    """
    import time

    import jax
    from jax.sharding import Mesh, NamedSharding, PartitionSpec
    from jax.experimental.shard_map import shard_map

    from concourse import bass2jax
    from concourse import mybir as mb

    arrs = _prep(inputs)
    nc = _make_nc(S)
    bass2jax.install_neuronx_cc_hook()

    partition_name = (
        nc.partition_id_tensor.name if nc.partition_id_tensor else None
    )
    in_names, out_names, out_avals, zero_outs = [], [], [], []
    for alloc in nc.m.functions[0].allocations:
        if not isinstance(alloc, mb.MemoryLocationSet):
            continue
        name = alloc.memorylocations[0].name
        if alloc.kind == "ExternalInput":
            if name != partition_name:
                in_names.append(name)
        elif alloc.kind == "ExternalOutput":
            out_names.append(name)
            shape = tuple(alloc.tensor_shape)
            dtype = mb.dt.np(alloc.dtype)
            out_avals.append(jax.core.ShapedArray(shape, dtype))
            zero_outs.append(np.zeros(shape, dtype))
    n_params = len(in_names)
    n_outs = len(out_avals)
    all_names = in_names + out_names
    if partition_name is not None:
        all_names = all_names + [partition_name]

    def _call(ins, zeros):
        operands = list(ins) + list(zeros)
        if partition_name is not None:
            operands.append(bass2jax.partition_id_tensor())
        return bass2jax._bass_exec_p.bind(
            *operands,
            out_avals=tuple(out_avals),
            in_names=tuple(all_names),
            out_names=tuple(out_names),
            lowering_input_output_aliases=(),
            sim_require_finite=True,
            sim_require_nnan=True,
            nc=nc,
        )

    def _body(*args):
        ins = list(args[:n_params])
        zeros = list(args[n_params:])
        outs = _call(ins, zeros)
        for _ in range(chain - 1):
            outs = _call(ins, list(outs))
        return tuple(outs)

    devices = jax.devices()[:B]
    mesh = Mesh(np.asarray(devices), ("core",))
    in_specs = (PartitionSpec("core"),) * (n_params + n_outs)
    out_specs = (PartitionSpec("core"),) * n_outs
    donate = tuple(range(n_params, n_params + n_outs))
    sharded = jax.jit(
        shard_map(_body, mesh=mesh, in_specs=in_specs, out_specs=out_specs,
                  check_rep=False),
        donate_argnums=donate,
        keep_unused=True,
    )

    per_core = [
        [arrs["x"][c] if n == "x" else arrs[n] for n in in_names[:n_params]]
        for c in range(B)
    ]
    concat_in = [
        np.concatenate([per_core[c][i] for c in range(B)], axis=0)
        for i in range(n_params)
    ]
    concat_zeros = [
        np.zeros((B * z.shape[0], *z.shape[1:]), z.dtype) for z in zero_outs
    ]

    shard = NamedSharding(mesh, PartitionSpec("core"))
    dev_in = [jax.device_put(a, shard) for a in concat_in]
    jax.block_until_ready(dev_in)

    times = []
    out_np = None
    for i in range(iters + 1):
        dev_zeros = [jax.device_put(z, shard) for z in concat_zeros]
        jax.block_until_ready(dev_zeros)
        t0 = time.perf_counter()
        outs = sharded(*dev_in, *dev_zeros)
        jax.block_until_ready(outs)
        dt = time.perf_counter() - t0
        if i == 0:
            idx = out_names.index("out")
            out_np = np.asarray(outs[idx]).reshape(B, S, EMB).astype(np.float32)
        else:
            times.append(dt)
    return out_np, times


# revision 38
# speedup vs baseline: 322.3928x; 322.3928x over previous
"""Single-head MHA (QKV proj + softmax attention) on 8 Trainium2 cores.

Problem: x[8, 4096, 256] f32; per-batch attention with per-head emb 256.
Sharding: data-parallel — one batch element per NeuronCore (8 cores).

Per-core algorithm (S=4096, E=256, P=128 partitions):
  - cast x to bf16, PE-transpose -> xT[d, s]
  - QT[e, s] = WqT.T @ xT (+bq), KT likewise, V[s, e] = xT.T @ WvT  (bf16)
  - per q-block of 1024 columns, loop k-tiles of 128 rows:
      S^T[k, q] = KT_slice.T @ QT   (4 matmuls of N=512, fp32 PSUM)
      E[k, q]   = exp(S^T / 16)     (one ScalarE op, scale fused, bf16 out)
      out[q, e]+= E_chunk.T @ V     (8 matmuls of N=256: E q-chunks are the
                                     stationary operand, so the output lands
                                     in [q, e] layout — no transposes needed)
      denom    += E                 (VectorE, fp32, two interleaved chains)
    softmax denominators: 8 tiny N=1 matmuls dall_chunk.T @ ones -> [q, 1]
    columns in PSUM; reciprocal; out = out_ps * recip + bv fused in one
    scalar_tensor_tensor per 128-row tile (softmax rows sum to 1, so
    attn @ (V + bv) = attn @ V + bv).

No running-max subtraction: scores/16 ~ N(0,1); max observed ~10.5, exp
stays well inside fp32/bf16 range.
"""

from contextlib import ExitStack

import numpy as np

import concourse.bass as bass
import concourse.tile as tile
from concourse import bacc
from concourse import mybir
from concourse import bass_utils
from concourse.masks import make_identity

P = 128          # partitions
EMB = 256        # head dim
S = 4096         # sequence length
B = 8            # batch == number of cores
QB = 1024        # q-block (free dim of S^T / E tiles; matmuls split into 512s)
MMN = 512        # max matmul free dim (one PSUM bank of fp32)

F32 = mybir.dt.float32
BF16 = mybir.dt.bfloat16
AF = mybir.ActivationFunctionType


def _build(nc: bass.Bass, s_len: int = S) -> None:
    """Emit the per-core program into `nc` (SPMD: same program all cores)."""
    x = nc.dram_tensor("x", (s_len, EMB), F32, kind="ExternalInput").ap()
    Wq = nc.dram_tensor("Wq", (EMB, EMB), F32, kind="ExternalInput").ap()
    bq = nc.dram_tensor("bq", (EMB,), F32, kind="ExternalInput").ap()
    Wk = nc.dram_tensor("Wk", (EMB, EMB), F32, kind="ExternalInput").ap()
    bk = nc.dram_tensor("bk", (EMB,), F32, kind="ExternalInput").ap()
    Wv = nc.dram_tensor("Wv", (EMB, EMB), F32, kind="ExternalInput").ap()
    bv = nc.dram_tensor("bv", (EMB,), F32, kind="ExternalInput").ap()
    out = nc.dram_tensor("out", (s_len, EMB), F32, kind="ExternalOutput").ap()

    n_st = s_len // P      # 128-row tiles of the sequence
    n_qb = s_len // QB     # q-blocks
    n_kt = s_len // P      # k-tiles
    n_qt = QB // P         # 128-row q-tiles per q-block
    scale = float(EMB) ** -0.5

    with tile.TileContext(nc) as tc, ExitStack() as ctx:
        consts = ctx.enter_context(tc.tile_pool(name="consts", bufs=1))
        persist = ctx.enter_context(tc.tile_pool(name="persist", bufs=1))
        stage = ctx.enter_context(tc.tile_pool(name="stage", bufs=6))
        work = ctx.enter_context(tc.tile_pool(name="work", bufs=5))
        outp = ctx.enter_context(tc.tile_pool(name="outp", bufs=6))
        # PSUM budget is exactly 8 banks: "mm" 2 slots x [128,1024]f32 (2
        # banks each) + "po" 2 slots x [128,1024]f32. Transpose outputs and
        # projection tiles share the "mm" slots.
        ps_mm = ctx.enter_context(tc.tile_pool(name="ps_mm", bufs=2, space="PSUM"))
        ps_acc = ctx.enter_context(tc.tile_pool(name="ps_acc", bufs=1, space="PSUM"))
        ps_tp = ps_mm

        # identity for PE transposes (all transposes run in f32; the cast to
        # bf16 happens on the PSUM->SBUF copy)
        idf = consts.tile([P, P], F32)
        make_identity(nc, idf)
        ones_f = consts.tile([P, 1], F32)
        nc.vector.memset(ones_f, 1.0)
        idb = consts.tile([P, P], BF16)
        nc.vector.tensor_copy(idb, idf)

        # biases: bq/bk as per-partition columns (e on partitions),
        # bv broadcast across partitions (added at the very end).
        bq_sb = consts.tile([P, 2], F32)
        nc.gpsimd.dma_start(bq_sb, bq.rearrange("(t p) -> p t", p=P))
        bk_sb = consts.tile([P, 2], F32)
        nc.gpsimd.dma_start(bk_sb, bk.rearrange("(t p) -> p t", p=P))
        bv_bc = consts.tile([P, EMB], F32)
        nc.gpsimd.dma_start(
            bv_bc,
            bass.AP(tensor=bv.tensor, offset=bv.offset, ap=[[0, P], list(bv.ap[0])]),
        )

        # ---- weights: load W[e,d], PE-transpose -> WT[d,e] (cast on copy) ----
        WT = {}
        for wname, wap in (("q", Wq), ("k", Wk), ("v", Wv)):
            wt0 = persist.tile([P, EMB], BF16, name=f"wt_{wname}_0")
            wt1 = persist.tile([P, EMB], BF16, name=f"wt_{wname}_1")
            WT[wname] = (wt0, wt1)
            for et in range(2):
                wst = stage.tile([P, EMB], F32, tag="wst")
                nc.sync.dma_start(wst, wap[et * P:(et + 1) * P, :])
                wbf = stage.tile([P, EMB], BF16, tag="wbf")
                nc.vector.tensor_copy(wbf, wst)
                for dc in range(2):
                    tp = ps_tp.tile([P, P], BF16, tag="mm")
                    nc.tensor.transpose(tp, wbf[:, dc * P:(dc + 1) * P], idb)
                    nc.scalar.copy(WT[wname][dc][:, et * P:(et + 1) * P], tp)

        # ---- x: load, PE-transpose -> xT[dc][d, s] (cast on copy) ----
        xT = [persist.tile([P, s_len], BF16, name=f"xT{dc}") for dc in range(2)]
        for st_i in range(n_st):
            xst = stage.tile([P, EMB], F32, tag="xst")
            nc.sync.dma_start(xst, x[st_i * P:(st_i + 1) * P, :])
            xbf = stage.tile([P, EMB], BF16, tag="xbf")
            nc.gpsimd.tensor_copy(xbf, xst)
            for dc in range(2):
                tp = ps_tp.tile([P, P], BF16, tag="mm")
                nc.tensor.transpose(tp, xbf[:, dc * P:(dc + 1) * P], idb)
                nc.scalar.copy(xT[dc][:, st_i * P:(st_i + 1) * P], tp)

        # ---- projections: QT/KT[e, s] (e on partitions), V[s, e] ----
        QT = [persist.tile([P, s_len], BF16, name=f"QT{t}") for t in range(2)]
        KT = [persist.tile([P, s_len], BF16, name=f"KT{t}") for t in range(2)]
        Vb = persist.tile([P, n_st, EMB], BF16, name="Vb")
        # K first, then V, then Q: the attention loop needs all of K/V but
        # only the first q-block of Q, so this order unblocks it earliest.
        for t in range(2):
            for sb in range(s_len // MMN):
                ssl = slice(sb * MMN, (sb + 1) * MMN)
                kps = ps_mm.tile([P, MMN], F32, tag="mm")
                nc.tensor.matmul(kps, WT["k"][0][:, t * P:(t + 1) * P],
                                 xT[0][:, ssl], start=True, stop=False)
                nc.tensor.matmul(kps, WT["k"][1][:, t * P:(t + 1) * P],
                                 xT[1][:, ssl], start=False, stop=True)
                nc.vector.tensor_scalar_add(KT[t][:, ssl], kps,
                                            bk_sb[:, t:t + 1])
        for st_i in range(n_st):
            vps = ps_mm.tile([P, EMB], F32, tag="mm")
            nc.tensor.matmul(vps, xT[0][:, st_i * P:(st_i + 1) * P], WT["v"][0],
                             start=True, stop=False)
            nc.tensor.matmul(vps, xT[1][:, st_i * P:(st_i + 1) * P], WT["v"][1],
                             start=False, stop=True)
            nc.vector.tensor_copy(Vb[:, st_i, :], vps)
        for sb in range(s_len // MMN):
            ssl = slice(sb * MMN, (sb + 1) * MMN)
            for t in range(2):
                qps = ps_mm.tile([P, MMN], F32, tag="mm")
                nc.tensor.matmul(qps, WT["q"][0][:, t * P:(t + 1) * P],
                                 xT[0][:, ssl], start=True, stop=False)
                nc.tensor.matmul(qps, WT["q"][1][:, t * P:(t + 1) * P],
                                 xT[1][:, ssl], start=False, stop=True)
                nc.vector.tensor_scalar_add(QT[t][:, ssl], qps,
                                            bq_sb[:, t:t + 1])

        # ---- attention ----
        ebf_chunks = n_qt  # 128-wide q-chunks of the exp tile
        for qb_i in range(n_qb):
            q0b = qb_i * QB
            # out accumulator in [q, e] layout: one PSUM tensor, 4 banks.
            # PV uses the exp tile E^T[k, q] as the *stationary* operand
            # (128-column q-chunks) and V[k, e] as the moving operand, so the
            # attention output lands directly in [q, e] — no transposes.
            out_ps = ps_acc.tile([P, n_qt, EMB], F32, tag="po",
                                 name=f"out_ps_{qb_i}")
            # two interleaved softmax-denominator accumulators, one summed on
            # the vector engine and one on gpsimd, so neither chain bottlenecks
            dacc = [work.tile([P, QB], F32, tag=f"dacc{i}", name=f"dacc{i}_{qb_i}")
                    for i in range(2)]
            e_prev = None
            for kt_i in range(n_kt):
                ksl = slice(kt_i * P, (kt_i + 1) * P)
                stp = ps_mm.tile([P, QB], F32, tag="mm")
                for t in range(2):
                    # lhsT (KT slice) is reused by both q-halves: one
                    # LDWEIGHTS serves two matmuls.
                    for qh in range(2):
                        hs = slice(qh * MMN, (qh + 1) * MMN)
                        nc.tensor.matmul(stp[:, hs], KT[t][:, ksl],
                                         QT[t][:, q0b + qh * MMN:
                                                q0b + (qh + 1) * MMN],
                                         start=(t == 0), stop=(t == 1))
                if e_prev is not None:
                    # PV for the previous k-tile: keeps PE busy while ACT
                    # computes this tile's exp (software pipelining).
                    kp = kt_i - 1
                    for j in range(n_qt):
                        # PSUM accumulation groups are bank-granular: two
                        # adjacent j-slices share a 2KB bank, so the group
                        # opens on the even slice and closes on the odd one.
                        nc.tensor.matmul(out_ps[:, j, :],
                                         e_prev[:, j * P:(j + 1) * P],
                                         Vb[:, kp, :],
                                         start=(kp == 0 and j % 2 == 0),
                                         stop=False)
                ebf = work.tile([P, QB], BF16, tag="E", bufs=8)
                nc.scalar.activation(ebf, stp, AF.Exp, scale=scale)
                idx = kt_i % 2
                eng = nc.vector
                if kt_i < 2:
                    eng.tensor_copy(dacc[idx], ebf)
                else:
                    eng.tensor_add(dacc[idx], dacc[idx], ebf)
                e_prev = ebf
            for j in range(n_qt):
                nc.tensor.matmul(out_ps[:, j, :],
                                 e_prev[:, j * P:(j + 1) * P],
                                 Vb[:, n_kt - 1, :],
                                 start=False, stop=(j % 2 == 1))

            dall = work.tile([P, QB], F32, tag="dall")
            nc.vector.tensor_add(dall, dacc[0], dacc[1])

            # softmax denominators via tiny N=1 matmuls: dn[:, j] =
            # dall_chunk_j.T @ ones -> [q, 1] columns, accumulated in a single
            # PSUM bank (group opens at j=0, closes at j=n_qt-1).
            dn_ps = ps_mm.tile([P, n_qt], F32, tag="mm", name=f"dn_{qb_i}")
            for j in range(n_qt):
                nc.tensor.matmul(dn_ps[:, j:j + 1],
                                 dall[:, j * P:(j + 1) * P], ones_f,
                                 start=(j == 0), stop=(j == n_qt - 1))
            recip = work.tile([P, n_qt], F32, tag="recip")
            nc.vector.reciprocal(recip, dn_ps)

            # finalize: out = out_ps*recip + bv, fused in one DVE op per tile
            for j in range(n_qt):
                res = outp.tile([P, EMB], F32, tag="res")
                nc.vector.scalar_tensor_tensor(
                    res, out_ps[:, j, :], recip[:, j:j + 1], bv_bc,
                    op0=mybir.AluOpType.mult, op1=mybir.AluOpType.add)
                q0 = q0b + j * P
                nc.sync.dma_start(out[q0:q0 + P, :], res)


def _make_nc(s_len: int = S) -> bass.Bass:
    # Bacc (not raw Bass): its compile() splits multi-sem waits and moves
    # matmul waits onto ldweights — HW allows at most one wait per inst.
    nc = bacc.Bacc("TRN2", target_bir_lowering=False, debug=False)
    _build(nc, s_len)
    nc.compile()
    return nc


def _prep(inputs: dict) -> dict:
    arrs = {k: np.ascontiguousarray(np.asarray(v, dtype=np.float32))
            for k, v in inputs.items()}
    assert arrs["x"].shape == (B, S, EMB), arrs["x"].shape
    return arrs


def run(inputs: dict):
    """Run on 8 NeuronCores. Returns (out[B,S,E] f32, BassKernelResults)."""
    arrs = _prep(inputs)
    nc = _make_nc(S)
    shared = {k: arrs[k] for k in ("Wq", "bq", "Wk", "bk", "Wv", "bv")}
    in_maps = [dict(shared, x=arrs["x"][i]) for i in range(B)]
    res = bass_utils.run_bass_kernel_spmd(nc, in_maps, core_ids=list(range(B)))
    out = np.stack([r["out"] for r in res.results], axis=0).astype(np.float32)
    return out, res


def kernel(**inputs) -> np.ndarray:
    out, _ = run(inputs)
    return out


def bench(inputs: dict, iters: int = 5, chain: int = 1):
    """Compile once, then time repeated executions with device-resident
    inputs (mirrors bass2jax.run_bass_via_pjrt's multi-core path).

    `chain` > 1 executes the NEFF that many times inside one XLA program
    (each call's outputs feed the next call's donated output buffers, which
    serializes them) so per-iteration device time can be extracted as a
    slope, amortizing the axon dispatch overhead.

    Returns (out[B,S,E] f32, list of per-call wall times in seconds).
    """
    import time

    import jax
    from jax.sharding import Mesh, NamedSharding, PartitionSpec
    from jax.experimental.shard_map import shard_map

    from concourse import bass2jax
    from concourse import mybir as mb

    arrs = _prep(inputs)
    nc = _make_nc(S)
    bass2jax.install_neuronx_cc_hook()

    partition_name = (
        nc.partition_id_tensor.name if nc.partition_id_tensor else None
    )
    in_names, out_names, out_avals, zero_outs = [], [], [], []
    for alloc in nc.m.functions[0].allocations:
        if not isinstance(alloc, mb.MemoryLocationSet):
            continue
        name = alloc.memorylocations[0].name
        if alloc.kind == "ExternalInput":
            if name != partition_name:
                in_names.append(name)
        elif alloc.kind == "ExternalOutput":
            out_names.append(name)
            shape = tuple(alloc.tensor_shape)
            dtype = mb.dt.np(alloc.dtype)
            out_avals.append(jax.core.ShapedArray(shape, dtype))
            zero_outs.append(np.zeros(shape, dtype))
    n_params = len(in_names)
    n_outs = len(out_avals)
    all_names = in_names + out_names
    if partition_name is not None:
        all_names = all_names + [partition_name]

    def _call(ins, zeros):
        operands = list(ins) + list(zeros)
        if partition_name is not None:
            operands.append(bass2jax.partition_id_tensor())
        return bass2jax._bass_exec_p.bind(
            *operands,
            out_avals=tuple(out_avals),
            in_names=tuple(all_names),
            out_names=tuple(out_names),
            lowering_input_output_aliases=(),
            sim_require_finite=True,
            sim_require_nnan=True,
            nc=nc,
        )

    def _body(*args):
        ins = list(args[:n_params])
        zeros = list(args[n_params:])
        outs = _call(ins, zeros)
        for _ in range(chain - 1):
            outs = _call(ins, list(outs))
        return tuple(outs)

    devices = jax.devices()[:B]
    mesh = Mesh(np.asarray(devices), ("core",))
    in_specs = (PartitionSpec("core"),) * (n_params + n_outs)
    out_specs = (PartitionSpec("core"),) * n_outs
    donate = tuple(range(n_params, n_params + n_outs))
    sharded = jax.jit(
        shard_map(_body, mesh=mesh, in_specs=in_specs, out_specs=out_specs,
                  check_rep=False),
        donate_argnums=donate,
        keep_unused=True,
    )

    per_core = [
        [arrs["x"][c] if n == "x" else arrs[n] for n in in_names[:n_params]]
        for c in range(B)
    ]
    concat_in = [
        np.concatenate([per_core[c][i] for c in range(B)], axis=0)
        for i in range(n_params)
    ]
    concat_zeros = [
        np.zeros((B * z.shape[0], *z.shape[1:]), z.dtype) for z in zero_outs
    ]

    shard = NamedSharding(mesh, PartitionSpec("core"))
    dev_in = [jax.device_put(a, shard) for a in concat_in]
    jax.block_until_ready(dev_in)

    times = []
    out_np = None
    for i in range(iters + 1):
        dev_zeros = [jax.device_put(z, shard) for z in concat_zeros]
        jax.block_until_ready(dev_zeros)
        t0 = time.perf_counter()
        outs = sharded(*dev_in, *dev_zeros)
        jax.block_until_ready(outs)
        dt = time.perf_counter() - t0
        if i == 0:
            idx = out_names.index("out")
            out_np = np.asarray(outs[idx]).reshape(B, S, EMB).astype(np.float32)
        else:
            times.append(dt)
    return out_np, times


# revision 42
# speedup vs baseline: 322.3991x; 1.0000x over previous
"""Single-head MHA (QKV proj + softmax attention) on 8 Trainium2 cores.

Problem: x[8, 4096, 256] f32; per-batch attention with per-head emb 256.
Sharding: data-parallel — one batch element per NeuronCore (8 cores).

Per-core algorithm (S=4096, E=256, P=128 partitions):
  - cast x to bf16, PE-transpose -> xT[d, s]
  - QT[e, s] = WqT.T @ xT (+bq), KT likewise, V[s, e] = xT.T @ WvT  (bf16)
  - per q-block of 1024 columns, loop k-tiles of 128 rows:
      S^T[k, q] = KT_slice.T @ QT   (4 matmuls of N=512, fp32 PSUM)
      E[k, q]   = exp(S^T / 16)     (one ScalarE op, scale fused, bf16 out)
      out[q, e]+= E_chunk.T @ V     (8 matmuls of N=256: E q-chunks are the
                                     stationary operand, so the output lands
                                     in [q, e] layout — no transposes needed)
      denom    += E                 (VectorE, fp32, two interleaved chains)
    softmax denominators: 8 tiny N=1 matmuls dall_chunk.T @ ones -> [q, 1]
    columns in PSUM; reciprocal; out = out_ps * recip + bv fused in one
    scalar_tensor_tensor per 128-row tile (softmax rows sum to 1, so
    attn @ (V + bv) = attn @ V + bv).

No running-max subtraction: scores/16 ~ N(0,1); max observed ~10.5, exp
stays well inside fp32/bf16 range.
"""

from contextlib import ExitStack

import numpy as np

import concourse.bass as bass
import concourse.tile as tile
from concourse import bacc
from concourse import mybir
from concourse import bass_utils
from concourse.masks import make_identity

P = 128          # partitions
EMB = 256        # head dim
S = 4096         # sequence length
B = 8            # batch == number of cores
QB = 1024        # q-block (free dim of S^T / E tiles; matmuls split into 512s)
MMN = 512        # max matmul free dim (one PSUM bank of fp32)

F32 = mybir.dt.float32
BF16 = mybir.dt.bfloat16
AF = mybir.ActivationFunctionType


def _build(nc: bass.Bass, s_len: int = S) -> None:
    """Emit the per-core program into `nc` (SPMD: same program all cores)."""
    x = nc.dram_tensor("x", (s_len, EMB), F32, kind="ExternalInput").ap()
    Wq = nc.dram_tensor("Wq", (EMB, EMB), F32, kind="ExternalInput").ap()
    bq = nc.dram_tensor("bq", (EMB,), F32, kind="ExternalInput").ap()
    Wk = nc.dram_tensor("Wk", (EMB, EMB), F32, kind="ExternalInput").ap()
    bk = nc.dram_tensor("bk", (EMB,), F32, kind="ExternalInput").ap()
    Wv = nc.dram_tensor("Wv", (EMB, EMB), F32, kind="ExternalInput").ap()
    bv = nc.dram_tensor("bv", (EMB,), F32, kind="ExternalInput").ap()
    out = nc.dram_tensor("out", (s_len, EMB), F32, kind="ExternalOutput").ap()

    n_st = s_len // P      # 128-row tiles of the sequence
    n_qb = s_len // QB     # q-blocks
    n_kt = s_len // P      # k-tiles
    n_qt = QB // P         # 128-row q-tiles per q-block
    scale = float(EMB) ** -0.5

    with tile.TileContext(nc) as tc, ExitStack() as ctx:
        consts = ctx.enter_context(tc.tile_pool(name="consts", bufs=1))
        persist = ctx.enter_context(tc.tile_pool(name="persist", bufs=1))
        stage = ctx.enter_context(tc.tile_pool(name="stage", bufs=6))
        work = ctx.enter_context(tc.tile_pool(name="work", bufs=5))
        outp = ctx.enter_context(tc.tile_pool(name="outp", bufs=6))
        # PSUM budget is exactly 8 banks: "mm" 2 slots x [128,1024]f32 (2
        # banks each) + "po" 2 slots x [128,1024]f32. Transpose outputs and
        # projection tiles share the "mm" slots.
        ps_mm = ctx.enter_context(tc.tile_pool(name="ps_mm", bufs=2, space="PSUM"))
        ps_acc = ctx.enter_context(tc.tile_pool(name="ps_acc", bufs=1, space="PSUM"))
        ps_tp = ps_mm

        # identity for PE transposes (all transposes run in f32; the cast to
        # bf16 happens on the PSUM->SBUF copy)
        idf = consts.tile([P, P], F32)
        make_identity(nc, idf)
        ones_f = consts.tile([P, 1], F32)
        nc.vector.memset(ones_f, 1.0)
        idb = consts.tile([P, P], BF16)
        nc.vector.tensor_copy(idb, idf)

        # biases: bq/bk as per-partition columns (e on partitions),
        # bv broadcast across partitions (added at the very end).
        bq_sb = consts.tile([P, 2], F32)
        nc.gpsimd.dma_start(bq_sb, bq.rearrange("(t p) -> p t", p=P))
        bk_sb = consts.tile([P, 2], F32)
        nc.gpsimd.dma_start(bk_sb, bk.rearrange("(t p) -> p t", p=P))
        bv_bc = consts.tile([P, EMB], F32)
        nc.gpsimd.dma_start(
            bv_bc,
            bass.AP(tensor=bv.tensor, offset=bv.offset, ap=[[0, P], list(bv.ap[0])]),
        )

        # ---- weights: load W[e,d], PE-transpose -> WT[d,e] (cast on copy) ----
        WT = {}
        for wname, wap in (("q", Wq), ("k", Wk), ("v", Wv)):
            wt0 = persist.tile([P, EMB], BF16, name=f"wt_{wname}_0")
            wt1 = persist.tile([P, EMB], BF16, name=f"wt_{wname}_1")
            WT[wname] = (wt0, wt1)
            for et in range(2):
                wst = stage.tile([P, EMB], F32, tag="wst")
                nc.sync.dma_start(wst, wap[et * P:(et + 1) * P, :])
                wbf = stage.tile([P, EMB], BF16, tag="wbf")
                nc.vector.tensor_copy(wbf, wst)
                for dc in range(2):
                    tp = ps_tp.tile([P, P], BF16, tag="mm")
                    nc.tensor.transpose(tp, wbf[:, dc * P:(dc + 1) * P], idb)
                    nc.scalar.copy(WT[wname][dc][:, et * P:(et + 1) * P], tp)

        # ---- x: load, PE-transpose -> xT[dc][d, s] (cast on copy) ----
        xT = [persist.tile([P, s_len], BF16, name=f"xT{dc}") for dc in range(2)]
        for st_i in range(n_st):
            xst = stage.tile([P, EMB], F32, tag="xst")
            nc.sync.dma_start(xst, x[st_i * P:(st_i + 1) * P, :])
            xbf = stage.tile([P, EMB], BF16, tag="xbf")
            nc.gpsimd.tensor_copy(xbf, xst)
            for dc in range(2):
                tp = ps_tp.tile([P, P], BF16, tag="mm")
                nc.tensor.transpose(tp, xbf[:, dc * P:(dc + 1) * P], idb)
                nc.scalar.copy(xT[dc][:, st_i * P:(st_i + 1) * P], tp)

        # ---- projections: QT/KT[e, s] (e on partitions), V[s, e] ----
        QT = [persist.tile([P, s_len], BF16, name=f"QT{t}") for t in range(2)]
        KT = [persist.tile([P, s_len], BF16, name=f"KT{t}") for t in range(2)]
        Vb = persist.tile([P, n_st, EMB], BF16, name="Vb")
        # K first, then V, then Q: the attention loop needs all of K/V but
        # only the first q-block of Q, so this order unblocks it earliest.
        for t in range(2):
            for sb in range(s_len // MMN):
                ssl = slice(sb * MMN, (sb + 1) * MMN)
                kps = ps_mm.tile([P, MMN], F32, tag="mm")
                nc.tensor.matmul(kps, WT["k"][0][:, t * P:(t + 1) * P],
                                 xT[0][:, ssl], start=True, stop=False)
                nc.tensor.matmul(kps, WT["k"][1][:, t * P:(t + 1) * P],
                                 xT[1][:, ssl], start=False, stop=True)
                nc.scalar.activation(KT[t][:, ssl], kps, AF.Identity,
                                     bias=bk_sb[:, t:t + 1], scale=1.0)
        for st_i in range(n_st):
            vps = ps_mm.tile([P, EMB], F32, tag="mm")
            nc.tensor.matmul(vps, xT[0][:, st_i * P:(st_i + 1) * P], WT["v"][0],
                             start=True, stop=False)
            nc.tensor.matmul(vps, xT[1][:, st_i * P:(st_i + 1) * P], WT["v"][1],
                             start=False, stop=True)
            nc.vector.tensor_copy(Vb[:, st_i, :], vps)
        for sb in range(s_len // MMN):
            ssl = slice(sb * MMN, (sb + 1) * MMN)
            for t in range(2):
                qps = ps_mm.tile([P, MMN], F32, tag="mm")
                nc.tensor.matmul(qps, WT["q"][0][:, t * P:(t + 1) * P],
                                 xT[0][:, ssl], start=True, stop=False)
                nc.tensor.matmul(qps, WT["q"][1][:, t * P:(t + 1) * P],
                                 xT[1][:, ssl], start=False, stop=True)
                nc.scalar.activation(QT[t][:, ssl], qps, AF.Identity,
                                     bias=bq_sb[:, t:t + 1], scale=1.0)

        # ---- attention ----
        ebf_chunks = n_qt  # 128-wide q-chunks of the exp tile
        for qb_i in range(n_qb):
            q0b = qb_i * QB
            # out accumulator in [q, e] layout: one PSUM tensor, 4 banks.
            # PV uses the exp tile E^T[k, q] as the *stationary* operand
            # (128-column q-chunks) and V[k, e] as the moving operand, so the
            # attention output lands directly in [q, e] — no transposes.
            out_ps = ps_acc.tile([P, n_qt, EMB], F32, tag="po",
                                 name=f"out_ps_{qb_i}")
            # two interleaved softmax-denominator accumulators, one summed on
            # the vector engine and one on gpsimd, so neither chain bottlenecks
            dacc = [work.tile([P, QB], F32, tag=f"dacc{i}", name=f"dacc{i}_{qb_i}")
                    for i in range(2)]
            e_prev = None
            for kt_i in range(n_kt):
                ksl = slice(kt_i * P, (kt_i + 1) * P)
                stp = ps_mm.tile([P, QB], F32, tag="mm")
                for t in range(2):
                    # lhsT (KT slice) is reused by both q-halves: one
                    # LDWEIGHTS serves two matmuls.
                    for qh in range(2):
                        hs = slice(qh * MMN, (qh + 1) * MMN)
                        nc.tensor.matmul(stp[:, hs], KT[t][:, ksl],
                                         QT[t][:, q0b + qh * MMN:
                                                q0b + (qh + 1) * MMN],
                                         start=(t == 0), stop=(t == 1))
                if e_prev is not None:
                    # PV for the previous k-tile: keeps PE busy while ACT
                    # computes this tile's exp (software pipelining).
                    kp = kt_i - 1
                    for j in range(n_qt):
                        # PSUM accumulation groups are bank-granular: two
                        # adjacent j-slices share a 2KB bank, so the group
                        # opens on the even slice and closes on the odd one.
                        nc.tensor.matmul(out_ps[:, j, :],
                                         e_prev[:, j * P:(j + 1) * P],
                                         Vb[:, kp, :],
                                         start=(kp == 0 and j % 2 == 0),
                                         stop=False)
                ebf = work.tile([P, QB], BF16, tag="E", bufs=8)
                nc.scalar.activation(ebf, stp, AF.Exp, scale=scale)
                idx = kt_i % 2
                eng = nc.vector
                if kt_i < 2:
                    eng.tensor_copy(dacc[idx], ebf)
                else:
                    eng.tensor_add(dacc[idx], dacc[idx], ebf)
                e_prev = ebf
            for j in range(n_qt):
                nc.tensor.matmul(out_ps[:, j, :],
                                 e_prev[:, j * P:(j + 1) * P],
                                 Vb[:, n_kt - 1, :],
                                 start=False, stop=(j % 2 == 1))

            dall = work.tile([P, QB], F32, tag="dall")
            nc.vector.tensor_add(dall, dacc[0], dacc[1])

            # softmax denominators via tiny N=1 matmuls: dn[:, j] =
            # dall_chunk_j.T @ ones -> [q, 1] columns, accumulated in a single
            # PSUM bank (group opens at j=0, closes at j=n_qt-1).
            dn_ps = ps_mm.tile([P, n_qt], F32, tag="mm", name=f"dn_{qb_i}")
            for j in range(n_qt):
                nc.tensor.matmul(dn_ps[:, j:j + 1],
                                 dall[:, j * P:(j + 1) * P], ones_f,
                                 start=(j == 0), stop=(j == n_qt - 1))
            recip = work.tile([P, n_qt], F32, tag="recip")
            nc.vector.reciprocal(recip, dn_ps)

            # finalize: out = out_ps*recip + bv, fused in one DVE op per tile
            for j in range(n_qt):
                res = outp.tile([P, EMB], F32, tag="res")
                nc.vector.scalar_tensor_tensor(
                    res, out_ps[:, j, :], recip[:, j:j + 1], bv_bc,
                    op0=mybir.AluOpType.mult, op1=mybir.AluOpType.add)
                q0 = q0b + j * P
                nc.sync.dma_start(out[q0:q0 + P, :], res)


def _make_nc(s_len: int = S) -> bass.Bass:
    # Bacc (not raw Bass): its compile() splits multi-sem waits and moves
    # matmul waits onto ldweights — HW allows at most one wait per inst.
    nc = bacc.Bacc("TRN2", target_bir_lowering=False, debug=False)
    _build(nc, s_len)
    nc.compile()
    return nc


def _prep(inputs: dict) -> dict:
    arrs = {k: np.ascontiguousarray(np.asarray(v, dtype=np.float32))
            for k, v in inputs.items()}
    assert arrs["x"].shape == (B, S, EMB), arrs["x"].shape
    return arrs


def run(inputs: dict):
    """Run on 8 NeuronCores. Returns (out[B,S,E] f32, BassKernelResults)."""
    arrs = _prep(inputs)
    nc = _make_nc(S)
    shared = {k: arrs[k] for k in ("Wq", "bq", "Wk", "bk", "Wv", "bv")}
    in_maps = [dict(shared, x=arrs["x"][i]) for i in range(B)]
    res = bass_utils.run_bass_kernel_spmd(nc, in_maps, core_ids=list(range(B)))
    out = np.stack([r["out"] for r in res.results], axis=0).astype(np.float32)
    return out, res


def kernel(**inputs) -> np.ndarray:
    out, _ = run(inputs)
    return out


def bench(inputs: dict, iters: int = 5, chain: int = 1):
    """Compile once, then time repeated executions with device-resident
    inputs (mirrors bass2jax.run_bass_via_pjrt's multi-core path).

    `chain` > 1 executes the NEFF that many times inside one XLA program
    (each call's outputs feed the next call's donated output buffers, which
    serializes them) so per-iteration device time can be extracted as a
    slope, amortizing the axon dispatch overhead.

    Returns (out[B,S,E] f32, list of per-call wall times in seconds).
    """
    import time

    import jax
    from jax.sharding import Mesh, NamedSharding, PartitionSpec
    from jax.experimental.shard_map import shard_map

    from concourse import bass2jax
    from concourse import mybir as mb

    arrs = _prep(inputs)
    nc = _make_nc(S)
    bass2jax.install_neuronx_cc_hook()

    partition_name = (
        nc.partition_id_tensor.name if nc.partition_id_tensor else None
    )
    in_names, out_names, out_avals, zero_outs = [], [], [], []
    for alloc in nc.m.functions[0].allocations:
        if not isinstance(alloc, mb.MemoryLocationSet):
            continue
        name = alloc.memorylocations[0].name
        if alloc.kind == "ExternalInput":
            if name != partition_name:
                in_names.append(name)
        elif alloc.kind == "ExternalOutput":
            out_names.append(name)
            shape = tuple(alloc.tensor_shape)
            dtype = mb.dt.np(alloc.dtype)
            out_avals.append(jax.core.ShapedArray(shape, dtype))
            zero_outs.append(np.zeros(shape, dtype))
    n_params = len(in_names)
    n_outs = len(out_avals)
    all_names = in_names + out_names
    if partition_name is not None:
        all_names = all_names + [partition_name]

    def _call(ins, zeros):
        operands = list(ins) + list(zeros)
        if partition_name is not None:
            operands.append(bass2jax.partition_id_tensor())
        return bass2jax._bass_exec_p.bind(
            *operands,
            out_avals=tuple(out_avals),
            in_names=tuple(all_names),
            out_names=tuple(out_names),
            lowering_input_output_aliases=(),
            sim_require_finite=True,
            sim_require_nnan=True,
            nc=nc,
        )

    def _body(*args):
        ins = list(args[:n_params])
        zeros = list(args[n_params:])
        outs = _call(ins, zeros)
        for _ in range(chain - 1):
            outs = _call(ins, list(outs))
        return tuple(outs)

    devices = jax.devices()[:B]
    mesh = Mesh(np.asarray(devices), ("core",))
    in_specs = (PartitionSpec("core"),) * (n_params + n_outs)
    out_specs = (PartitionSpec("core"),) * n_outs
    donate = tuple(range(n_params, n_params + n_outs))
    sharded = jax.jit(
        shard_map(_body, mesh=mesh, in_specs=in_specs, out_specs=out_specs,
                  check_rep=False),
        donate_argnums=donate,
        keep_unused=True,
    )

    per_core = [
        [arrs["x"][c] if n == "x" else arrs[n] for n in in_names[:n_params]]
        for c in range(B)
    ]
    concat_in = [
        np.concatenate([per_core[c][i] for c in range(B)], axis=0)
        for i in range(n_params)
    ]
    concat_zeros = [
        np.zeros((B * z.shape[0], *z.shape[1:]), z.dtype) for z in zero_outs
    ]

    shard = NamedSharding(mesh, PartitionSpec("core"))
    dev_in = [jax.device_put(a, shard) for a in concat_in]
    jax.block_until_ready(dev_in)

    times = []
    out_np = None
    for i in range(iters + 1):
        dev_zeros = [jax.device_put(z, shard) for z in concat_zeros]
        jax.block_until_ready(dev_zeros)
        t0 = time.perf_counter()
        outs = sharded(*dev_in, *dev_zeros)
        jax.block_until_ready(outs)
        dt = time.perf_counter() - t0
        if i == 0:
            idx = out_names.index("out")
            out_np = np.asarray(outs[idx]).reshape(B, S, EMB).astype(np.float32)
        else:
            times.append(dt)
    return out_np, times


# revision 45
# speedup vs baseline: 326.0047x; 1.0112x over previous
"""Single-head MHA (QKV proj + softmax attention) on 8 Trainium2 cores.

Problem: x[8, 4096, 256] f32; per-batch attention with per-head emb 256.
Sharding: data-parallel — one batch element per NeuronCore (8 cores).

Per-core algorithm (S=4096, E=256, P=128 partitions):
  - cast x to bf16, PE-transpose -> xT[d, s]
  - QT[e, s] = WqT.T @ xT (+bq), KT likewise, V[s, e] = xT.T @ WvT  (bf16)
  - per q-block of 1024 columns, loop k-tiles of 128 rows:
      S^T[k, q] = KT_slice.T @ QT   (4 matmuls of N=512, fp32 PSUM)
      E[k, q]   = exp(S^T / 16)     (one ScalarE op, scale fused, bf16 out)
      out[q, e]+= E_chunk.T @ V     (8 matmuls of N=256: E q-chunks are the
                                     stationary operand, so the output lands
                                     in [q, e] layout — no transposes needed)
      denom    += E                 (VectorE, fp32, two interleaved chains)
    softmax denominators: 8 tiny N=1 matmuls dall_chunk.T @ ones -> [q, 1]
    columns in PSUM; reciprocal; out = out_ps * recip + bv fused in one
    scalar_tensor_tensor per 128-row tile (softmax rows sum to 1, so
    attn @ (V + bv) = attn @ V + bv).

No running-max subtraction: scores/16 ~ N(0,1); max observed ~10.5, exp
stays well inside fp32/bf16 range.
"""

from contextlib import ExitStack

import numpy as np

import concourse.bass as bass
import concourse.tile as tile
from concourse import bacc
from concourse import mybir
from concourse import bass_utils
from concourse.masks import make_identity

P = 128          # partitions
EMB = 256        # head dim
S = 4096         # sequence length
B = 8            # batch == number of cores
QB = 1024        # q-block (free dim of S^T / E tiles; matmuls split into 512s)
MMN = 512        # max matmul free dim (one PSUM bank of fp32)

F32 = mybir.dt.float32
BF16 = mybir.dt.bfloat16
AF = mybir.ActivationFunctionType


def _build(nc: bass.Bass, s_len: int = S) -> None:
    """Emit the per-core program into `nc` (SPMD: same program all cores)."""
    x = nc.dram_tensor("x", (s_len, EMB), F32, kind="ExternalInput").ap()
    Wq = nc.dram_tensor("Wq", (EMB, EMB), F32, kind="ExternalInput").ap()
    bq = nc.dram_tensor("bq", (EMB,), F32, kind="ExternalInput").ap()
    Wk = nc.dram_tensor("Wk", (EMB, EMB), F32, kind="ExternalInput").ap()
    bk = nc.dram_tensor("bk", (EMB,), F32, kind="ExternalInput").ap()
    Wv = nc.dram_tensor("Wv", (EMB, EMB), F32, kind="ExternalInput").ap()
    bv = nc.dram_tensor("bv", (EMB,), F32, kind="ExternalInput").ap()
    out = nc.dram_tensor("out", (s_len, EMB), F32, kind="ExternalOutput").ap()

    n_st = s_len // P      # 128-row tiles of the sequence
    n_qb = s_len // QB     # q-blocks
    n_kt = s_len // P      # k-tiles
    n_qt = QB // P         # 128-row q-tiles per q-block
    scale = float(EMB) ** -0.5

    with tile.TileContext(nc) as tc, ExitStack() as ctx:
        consts = ctx.enter_context(tc.tile_pool(name="consts", bufs=1))
        persist = ctx.enter_context(tc.tile_pool(name="persist", bufs=1))
        stage = ctx.enter_context(tc.tile_pool(name="stage", bufs=6))
        work = ctx.enter_context(tc.tile_pool(name="work", bufs=5))
        outp = ctx.enter_context(tc.tile_pool(name="outp", bufs=6))
        # PSUM budget is exactly 8 banks: "mm" 2 slots x [128,1024]f32 (2
        # banks each) + "po" 2 slots x [128,1024]f32. Transpose outputs and
        # projection tiles share the "mm" slots.
        ps_mm = ctx.enter_context(tc.tile_pool(name="ps_mm", bufs=2, space="PSUM"))
        ps_acc = ctx.enter_context(tc.tile_pool(name="ps_acc", bufs=1, space="PSUM"))
        ps_tp = ps_mm

        # identity for PE transposes (all transposes run in f32; the cast to
        # bf16 happens on the PSUM->SBUF copy)
        idf = consts.tile([P, P], F32)
        make_identity(nc, idf)
        ones_f = consts.tile([P, 1], F32)
        nc.vector.memset(ones_f, 1.0)
        idb = consts.tile([P, P], BF16)
        nc.vector.tensor_copy(idb, idf)

        # biases: bq/bk as per-partition columns (e on partitions),
        # bv broadcast across partitions (added at the very end).
        bq_sb = consts.tile([P, 2], F32)
        nc.gpsimd.dma_start(bq_sb, bq.rearrange("(t p) -> p t", p=P))
        bk_sb = consts.tile([P, 2], F32)
        nc.gpsimd.dma_start(bk_sb, bk.rearrange("(t p) -> p t", p=P))
        bv_bc = consts.tile([P, EMB], F32)
        nc.gpsimd.dma_start(
            bv_bc,
            bass.AP(tensor=bv.tensor, offset=bv.offset, ap=[[0, P], list(bv.ap[0])]),
        )

        # ---- weights: load W[e,d], PE-transpose -> WT[d,e] (cast on copy) ----
        WT = {}
        for wname, wap in (("q", Wq), ("k", Wk), ("v", Wv)):
            wt0 = persist.tile([P, EMB], BF16, name=f"wt_{wname}_0")
            wt1 = persist.tile([P, EMB], BF16, name=f"wt_{wname}_1")
            WT[wname] = (wt0, wt1)
            for et in range(2):
                wst = stage.tile([P, EMB], F32, tag="wst")
                nc.sync.dma_start(wst, wap[et * P:(et + 1) * P, :])
                wbf = stage.tile([P, EMB], BF16, tag="wbf")
                nc.vector.tensor_copy(wbf, wst)
                for dc in range(2):
                    tp = ps_tp.tile([P, P], BF16, tag="mm")
                    nc.tensor.transpose(tp, wbf[:, dc * P:(dc + 1) * P], idb)
                    nc.scalar.copy(WT[wname][dc][:, et * P:(et + 1) * P], tp)

        # ---- x: load, PE-transpose -> xT[dc][d, s] (cast on copy) ----
        xT = [persist.tile([P, s_len], BF16, name=f"xT{dc}") for dc in range(2)]
        for st_i in range(n_st):
            xst = stage.tile([P, EMB], F32, tag="xst")
            nc.sync.dma_start(xst, x[st_i * P:(st_i + 1) * P, :])
            xbf = stage.tile([P, EMB], BF16, tag="xbf")
            nc.gpsimd.tensor_copy(xbf, xst)
            for dc in range(2):
                tp = ps_tp.tile([P, P], BF16, tag="mm")
                nc.tensor.transpose(tp, xbf[:, dc * P:(dc + 1) * P], idb)
                nc.scalar.copy(xT[dc][:, st_i * P:(st_i + 1) * P], tp)

        # ---- projections: QT/KT[e, s] (e on partitions), V[s, e] ----
        QT = [persist.tile([P, s_len], BF16, name=f"QT{t}") for t in range(2)]
        KT = [persist.tile([P, s_len], BF16, name=f"KT{t}") for t in range(2)]
        Vb = persist.tile([P, n_st, EMB], BF16, name="Vb")
        # K first, then V, then Q: the attention loop needs all of K/V but
        # only the first q-block of Q, so this order unblocks it earliest.
        for t in range(2):
            for sb in range(s_len // MMN):
                ssl = slice(sb * MMN, (sb + 1) * MMN)
                kps = ps_mm.tile([P, MMN], F32, tag="mm")
                nc.tensor.matmul(kps, WT["k"][0][:, t * P:(t + 1) * P],
                                 xT[0][:, ssl], start=True, stop=False)
                nc.tensor.matmul(kps, WT["k"][1][:, t * P:(t + 1) * P],
                                 xT[1][:, ssl], start=False, stop=True)
                nc.scalar.activation(KT[t][:, ssl], kps, AF.Identity,
                                     bias=bk_sb[:, t:t + 1], scale=1.0)
        for st_i in range(n_st):
            vps = ps_mm.tile([P, EMB], F32, tag="mm")
            nc.tensor.matmul(vps, xT[0][:, st_i * P:(st_i + 1) * P], WT["v"][0],
                             start=True, stop=False)
            nc.tensor.matmul(vps, xT[1][:, st_i * P:(st_i + 1) * P], WT["v"][1],
                             start=False, stop=True)
            nc.vector.tensor_copy(Vb[:, st_i, :], vps)
        for sb in range(s_len // MMN):
            ssl = slice(sb * MMN, (sb + 1) * MMN)
            for t in range(2):
                qps = ps_mm.tile([P, MMN], F32, tag="mm")
                nc.tensor.matmul(qps, WT["q"][0][:, t * P:(t + 1) * P],
                                 xT[0][:, ssl], start=True, stop=False)
                nc.tensor.matmul(qps, WT["q"][1][:, t * P:(t + 1) * P],
                                 xT[1][:, ssl], start=False, stop=True)
                nc.scalar.activation(QT[t][:, ssl], qps, AF.Identity,
                                     bias=bq_sb[:, t:t + 1], scale=1.0)

        # ---- attention ----
        ebf_chunks = n_qt  # 128-wide q-chunks of the exp tile
        for qb_i in range(n_qb):
            q0b = qb_i * QB
            # out accumulator in [q, e] layout: one PSUM tensor, 4 banks.
            # PV uses the exp tile E^T[k, q] as the *stationary* operand
            # (128-column q-chunks) and V[k, e] as the moving operand, so the
            # attention output lands directly in [q, e] — no transposes.
            out_ps = ps_acc.tile([P, n_qt, EMB], F32, tag="po",
                                 name=f"out_ps_{qb_i}")
            # two interleaved softmax-denominator accumulators, one summed on
            # the vector engine and one on gpsimd, so neither chain bottlenecks
            dacc = [work.tile([P, QB], F32, tag=f"dacc{i}", name=f"dacc{i}_{qb_i}")
                    for i in range(2)]
            elist = []
            for kt_i in range(n_kt):
                ksl = slice(kt_i * P, (kt_i + 1) * P)
                stp = ps_mm.tile([P, QB], F32, tag="mm")
                for t in range(2):
                    # lhsT (KT slice) is reused by both q-halves: one
                    # LDWEIGHTS serves two matmuls.
                    for qh in range(2):
                        hs = slice(qh * MMN, (qh + 1) * MMN)
                        nc.tensor.matmul(stp[:, hs], KT[t][:, ksl],
                                         QT[t][:, q0b + qh * MMN:
                                                q0b + (qh + 1) * MMN],
                                         start=(t == 0), stop=(t == 1))
                if kt_i >= 2:
                    # PV lags two k-tiles: extra slack on the exp dependency.
                    kp = kt_i - 2
                    for j in range(n_qt):
                        # PSUM accumulation groups are bank-granular: two
                        # adjacent j-slices share a 2KB bank, so the group
                        # opens on the even slice and closes on the odd one.
                        nc.tensor.matmul(out_ps[:, j, :],
                                         elist[kp][:, j * P:(j + 1) * P],
                                         Vb[:, kp, :],
                                         start=(kp == 0 and j % 2 == 0),
                                         stop=False)
                ebf = work.tile([P, QB], BF16, tag="E", bufs=8)
                nc.scalar.activation(ebf, stp, AF.Exp, scale=scale)
                idx = kt_i % 2
                eng = nc.vector
                if kt_i < 2:
                    eng.tensor_copy(dacc[idx], ebf)
                else:
                    eng.tensor_add(dacc[idx], dacc[idx], ebf)
                elist.append(ebf)
            for kp in (n_kt - 2, n_kt - 1):
                for j in range(n_qt):
                    nc.tensor.matmul(out_ps[:, j, :],
                                     elist[kp][:, j * P:(j + 1) * P],
                                     Vb[:, kp, :],
                                     start=False, stop=(kp == n_kt - 1 and j % 2 == 1))

            dall = work.tile([P, QB], F32, tag="dall")
            nc.vector.tensor_add(dall, dacc[0], dacc[1])

            # softmax denominators via tiny N=1 matmuls: dn[:, j] =
            # dall_chunk_j.T @ ones -> [q, 1] columns, accumulated in a single
            # PSUM bank (group opens at j=0, closes at j=n_qt-1).
            dn_ps = ps_mm.tile([P, n_qt], F32, tag="mm", name=f"dn_{qb_i}")
            for j in range(n_qt):
                nc.tensor.matmul(dn_ps[:, j:j + 1],
                                 dall[:, j * P:(j + 1) * P], ones_f,
                                 start=(j == 0), stop=(j == n_qt - 1))
            recip = work.tile([P, n_qt], F32, tag="recip")
            nc.vector.reciprocal(recip, dn_ps)

            # finalize: out = out_ps*recip + bv, fused in one DVE op per tile
            for j in range(n_qt):
                res = outp.tile([P, EMB], F32, tag="res")
                nc.vector.scalar_tensor_tensor(
                    res, out_ps[:, j, :], recip[:, j:j + 1], bv_bc,
                    op0=mybir.AluOpType.mult, op1=mybir.AluOpType.add)
                q0 = q0b + j * P
                nc.sync.dma_start(out[q0:q0 + P, :], res)


def _make_nc(s_len: int = S) -> bass.Bass:
    # Bacc (not raw Bass): its compile() splits multi-sem waits and moves
    # matmul waits onto ldweights — HW allows at most one wait per inst.
    nc = bacc.Bacc("TRN2", target_bir_lowering=False, debug=False)
    _build(nc, s_len)
    nc.compile()
    return nc


def _prep(inputs: dict) -> dict:
    arrs = {k: np.ascontiguousarray(np.asarray(v, dtype=np.float32))
            for k, v in inputs.items()}
    assert arrs["x"].shape == (B, S, EMB), arrs["x"].shape
    return arrs


def run(inputs: dict):
    """Run on 8 NeuronCores. Returns (out[B,S,E] f32, BassKernelResults)."""
    arrs = _prep(inputs)
    nc = _make_nc(S)
    shared = {k: arrs[k] for k in ("Wq", "bq", "Wk", "bk", "Wv", "bv")}
    in_maps = [dict(shared, x=arrs["x"][i]) for i in range(B)]
    res = bass_utils.run_bass_kernel_spmd(nc, in_maps, core_ids=list(range(B)))
    out = np.stack([r["out"] for r in res.results], axis=0).astype(np.float32)
    return out, res


def kernel(**inputs) -> np.ndarray:
    out, _ = run(inputs)
    return out


def bench(inputs: dict, iters: int = 5, chain: int = 1):
    """Compile once, then time repeated executions with device-resident
    inputs (mirrors bass2jax.run_bass_via_pjrt's multi-core path).

    `chain` > 1 executes the NEFF that many times inside one XLA program
    (each call's outputs feed the next call's donated output buffers, which
    serializes them) so per-iteration device time can be extracted as a
    slope, amortizing the axon dispatch overhead.

    Returns (out[B,S,E] f32, list of per-call wall times in seconds).
    """
    import time

    import jax
    from jax.sharding import Mesh, NamedSharding, PartitionSpec
    from jax.experimental.shard_map import shard_map

    from concourse import bass2jax
    from concourse import mybir as mb

    arrs = _prep(inputs)
    nc = _make_nc(S)
    bass2jax.install_neuronx_cc_hook()

    partition_name = (
        nc.partition_id_tensor.name if nc.partition_id_tensor else None
    )
    in_names, out_names, out_avals, zero_outs = [], [], [], []
    for alloc in nc.m.functions[0].allocations:
        if not isinstance(alloc, mb.MemoryLocationSet):
            continue
        name = alloc.memorylocations[0].name
        if alloc.kind == "ExternalInput":
            if name != partition_name:
                in_names.append(name)
        elif alloc.kind == "ExternalOutput":
            out_names.append(name)
            shape = tuple(alloc.tensor_shape)
            dtype = mb.dt.np(alloc.dtype)
            out_avals.append(jax.core.ShapedArray(shape, dtype))
            zero_outs.append(np.zeros(shape, dtype))
    n_params = len(in_names)
    n_outs = len(out_avals)
    all_names = in_names + out_names
    if partition_name is not None:
        all_names = all_names + [partition_name]

    def _call(ins, zeros):
        operands = list(ins) + list(zeros)
        if partition_name is not None:
            operands.append(bass2jax.partition_id_tensor())
        return bass2jax._bass_exec_p.bind(
            *operands,
            out_avals=tuple(out_avals),
            in_names=tuple(all_names),
            out_names=tuple(out_names),
            lowering_input_output_aliases=(),
            sim_require_finite=True,
            sim_require_nnan=True,
            nc=nc,
        )

    def _body(*args):
        ins = list(args[:n_params])
        zeros = list(args[n_params:])
        outs = _call(ins, zeros)
        for _ in range(chain - 1):
            outs = _call(ins, list(outs))
        return tuple(outs)

    devices = jax.devices()[:B]
    mesh = Mesh(np.asarray(devices), ("core",))
    in_specs = (PartitionSpec("core"),) * (n_params + n_outs)
    out_specs = (PartitionSpec("core"),) * n_outs
    donate = tuple(range(n_params, n_params + n_outs))
    sharded = jax.jit(
        shard_map(_body, mesh=mesh, in_specs=in_specs, out_specs=out_specs,
                  check_rep=False),
        donate_argnums=donate,
        keep_unused=True,
    )

    per_core = [
        [arrs["x"][c] if n == "x" else arrs[n] for n in in_names[:n_params]]
        for c in range(B)
    ]
    concat_in = [
        np.concatenate([per_core[c][i] for c in range(B)], axis=0)
        for i in range(n_params)
    ]
    concat_zeros = [
        np.zeros((B * z.shape[0], *z.shape[1:]), z.dtype) for z in zero_outs
    ]

    shard = NamedSharding(mesh, PartitionSpec("core"))
    dev_in = [jax.device_put(a, shard) for a in concat_in]
    jax.block_until_ready(dev_in)

    times = []
    out_np = None
    for i in range(iters + 1):
        dev_zeros = [jax.device_put(z, shard) for z in concat_zeros]
        jax.block_until_ready(dev_zeros)
        t0 = time.perf_counter()
        outs = sharded(*dev_in, *dev_zeros)
        jax.block_until_ready(outs)
        dt = time.perf_counter() - t0
        if i == 0:
            idx = out_names.index("out")
            out_np = np.asarray(outs[idx]).reshape(B, S, EMB).astype(np.float32)
        else:
            times.append(dt)
    return out_np, times


# revision 48
# speedup vs baseline: 331.5196x; 1.0169x over previous
"""Single-head MHA (QKV proj + softmax attention) on 8 Trainium2 cores.

Problem: x[8, 4096, 256] f32; per-batch attention with per-head emb 256.
Sharding: data-parallel — one batch element per NeuronCore (8 cores).

Per-core algorithm (S=4096, E=256, P=128 partitions):
  - cast x to bf16, PE-transpose -> xT[d, s]
  - QT[e, s] = WqT.T @ xT (+bq), KT likewise, V[s, e] = xT.T @ WvT  (bf16)
  - per q-block of 1024 columns, loop k-tiles of 128 rows:
      S^T[k, q] = KT_slice.T @ QT   (4 matmuls of N=512, fp32 PSUM)
      E[k, q]   = exp(S^T / 16)     (one ScalarE op, scale fused, bf16 out)
      out[q, e]+= E_chunk.T @ V     (8 matmuls of N=256: E q-chunks are the
                                     stationary operand, so the output lands
                                     in [q, e] layout — no transposes needed)
      denom    += E                 (VectorE, fp32, two interleaved chains)
    softmax denominators: 8 tiny N=1 matmuls dall_chunk.T @ ones -> [q, 1]
    columns in PSUM; reciprocal; out = out_ps * recip + bv fused in one
    scalar_tensor_tensor per 128-row tile (softmax rows sum to 1, so
    attn @ (V + bv) = attn @ V + bv).

No running-max subtraction: scores/16 ~ N(0,1); max observed ~10.5, exp
stays well inside fp32/bf16 range.
"""

from contextlib import ExitStack

import numpy as np

import concourse.bass as bass
import concourse.tile as tile
from concourse import bacc
from concourse import mybir
from concourse import bass_utils
from concourse.masks import make_identity

P = 128          # partitions
EMB = 256        # head dim
S = 4096         # sequence length
B = 8            # batch == number of cores
QB = 1024        # q-block (free dim of S^T / E tiles; matmuls split into 512s)
MMN = 512        # max matmul free dim (one PSUM bank of fp32)

F32 = mybir.dt.float32
BF16 = mybir.dt.bfloat16
AF = mybir.ActivationFunctionType


def _build(nc: bass.Bass, s_len: int = S) -> None:
    """Emit the per-core program into `nc` (SPMD: same program all cores)."""
    x = nc.dram_tensor("x", (s_len, EMB), F32, kind="ExternalInput").ap()
    Wq = nc.dram_tensor("Wq", (EMB, EMB), F32, kind="ExternalInput").ap()
    bq = nc.dram_tensor("bq", (EMB,), F32, kind="ExternalInput").ap()
    Wk = nc.dram_tensor("Wk", (EMB, EMB), F32, kind="ExternalInput").ap()
    bk = nc.dram_tensor("bk", (EMB,), F32, kind="ExternalInput").ap()
    Wv = nc.dram_tensor("Wv", (EMB, EMB), F32, kind="ExternalInput").ap()
    bv = nc.dram_tensor("bv", (EMB,), F32, kind="ExternalInput").ap()
    out = nc.dram_tensor("out", (s_len, EMB), F32, kind="ExternalOutput").ap()

    n_st = s_len // P      # 128-row tiles of the sequence
    n_qb = s_len // QB     # q-blocks
    n_kt = s_len // P      # k-tiles
    n_qt = QB // P         # 128-row q-tiles per q-block
    scale = float(EMB) ** -0.5

    with tile.TileContext(nc) as tc, ExitStack() as ctx:
        consts = ctx.enter_context(tc.tile_pool(name="consts", bufs=1))
        persist = ctx.enter_context(tc.tile_pool(name="persist", bufs=1))
        stage = ctx.enter_context(tc.tile_pool(name="stage", bufs=6))
        work = ctx.enter_context(tc.tile_pool(name="work", bufs=5))
        outp = ctx.enter_context(tc.tile_pool(name="outp", bufs=6))
        # PSUM budget is exactly 8 banks: "mm" 2 slots x [128,1024]f32 (2
        # banks each) + "po" 2 slots x [128,1024]f32. Transpose outputs and
        # projection tiles share the "mm" slots.
        ps_mm = ctx.enter_context(tc.tile_pool(name="ps_mm", bufs=2, space="PSUM"))
        ps_acc = ctx.enter_context(tc.tile_pool(name="ps_acc", bufs=1, space="PSUM"))
        ps_tp = ps_mm

        # identity for PE transposes (all transposes run in f32; the cast to
        # bf16 happens on the PSUM->SBUF copy)
        idf = consts.tile([P, P], F32)
        make_identity(nc, idf)
        ones_f = consts.tile([P, 1], F32)
        nc.vector.memset(ones_f, 1.0)
        idb = consts.tile([P, P], BF16)
        nc.vector.tensor_copy(idb, idf)

        # biases: bq/bk as per-partition columns (e on partitions),
        # bv broadcast across partitions (added at the very end).
        bq_sb = consts.tile([P, 2], F32)
        nc.gpsimd.dma_start(bq_sb, bq.rearrange("(t p) -> p t", p=P))
        bk_sb = consts.tile([P, 2], F32)
        nc.gpsimd.dma_start(bk_sb, bk.rearrange("(t p) -> p t", p=P))
        bv_bc = consts.tile([P, EMB], F32)
        nc.gpsimd.dma_start(
            bv_bc,
            bass.AP(tensor=bv.tensor, offset=bv.offset, ap=[[0, P], list(bv.ap[0])]),
        )

        # ---- weights: load W[e,d], PE-transpose -> WT[d,e] (cast on copy) ----
        WT = {}
        for wname, wap in (("q", Wq), ("k", Wk), ("v", Wv)):
            wt0 = persist.tile([P, EMB], BF16, name=f"wt_{wname}_0")
            wt1 = persist.tile([P, EMB], BF16, name=f"wt_{wname}_1")
            WT[wname] = (wt0, wt1)
            for et in range(2):
                wst = stage.tile([P, EMB], F32, tag="wst")
                nc.sync.dma_start(wst, wap[et * P:(et + 1) * P, :])
                wbf = stage.tile([P, EMB], BF16, tag="wbf")
                nc.vector.tensor_copy(wbf, wst)
                for dc in range(2):
                    tp = ps_tp.tile([P, P], BF16, tag="mm")
                    nc.tensor.transpose(tp, wbf[:, dc * P:(dc + 1) * P], idb)
                    nc.scalar.copy(WT[wname][dc][:, et * P:(et + 1) * P], tp)

        # ---- x: load, PE-transpose -> xT[dc][d, s] (cast on copy) ----
        xT = [persist.tile([P, s_len], BF16, name=f"xT{dc}") for dc in range(2)]
        for st_i in range(n_st):
            xst = stage.tile([P, EMB], F32, tag="xst")
            nc.sync.dma_start(xst, x[st_i * P:(st_i + 1) * P, :])
            xbf = stage.tile([P, EMB], BF16, tag="xbf")
            nc.gpsimd.tensor_copy(xbf, xst)
            for dc in range(2):
                tp = ps_tp.tile([P, P], BF16, tag="mm")
                nc.tensor.transpose(tp, xbf[:, dc * P:(dc + 1) * P], idb)
                nc.scalar.copy(xT[dc][:, st_i * P:(st_i + 1) * P], tp)

        # ---- projections: QT/KT[e, s] (e on partitions), V[s, e] ----
        QT = [persist.tile([P, s_len], BF16, name=f"QT{t}") for t in range(2)]
        KT = [persist.tile([P, s_len], BF16, name=f"KT{t}") for t in range(2)]
        Vb = persist.tile([P, n_st, EMB], BF16, name="Vb")
        # K first, then V, then Q: the attention loop needs all of K/V but
        # only the first q-block of Q, so this order unblocks it earliest.
        for t in range(2):
            for sb in range(s_len // MMN):
                ssl = slice(sb * MMN, (sb + 1) * MMN)
                kps = ps_mm.tile([P, MMN], F32, tag="mm")
                nc.tensor.matmul(kps, WT["k"][0][:, t * P:(t + 1) * P],
                                 xT[0][:, ssl], start=True, stop=False)
                nc.tensor.matmul(kps, WT["k"][1][:, t * P:(t + 1) * P],
                                 xT[1][:, ssl], start=False, stop=True)
                nc.scalar.activation(KT[t][:, ssl], kps, AF.Identity,
                                     bias=bk_sb[:, t:t + 1], scale=1.0)
        for st_i in range(n_st):
            vps = ps_mm.tile([P, EMB], F32, tag="mm")
            nc.tensor.matmul(vps, xT[0][:, st_i * P:(st_i + 1) * P], WT["v"][0],
                             start=True, stop=False)
            nc.tensor.matmul(vps, xT[1][:, st_i * P:(st_i + 1) * P], WT["v"][1],
                             start=False, stop=True)
            nc.vector.tensor_copy(Vb[:, st_i, :], vps)
        for sb in range(s_len // MMN):
            ssl = slice(sb * MMN, (sb + 1) * MMN)
            for t in range(2):
                qps = ps_mm.tile([P, MMN], F32, tag="mm")
                nc.tensor.matmul(qps, WT["q"][0][:, t * P:(t + 1) * P],
                                 xT[0][:, ssl], start=True, stop=False)
                nc.tensor.matmul(qps, WT["q"][1][:, t * P:(t + 1) * P],
                                 xT[1][:, ssl], start=False, stop=True)
                nc.scalar.activation(QT[t][:, ssl], qps, AF.Identity,
                                     bias=bq_sb[:, t:t + 1], scale=1.0)

        # ---- attention ----
        ebf_chunks = n_qt  # 128-wide q-chunks of the exp tile
        for qb_i in range(n_qb):
            q0b = qb_i * QB
            # out accumulator in [q, e] layout: one PSUM tensor, 4 banks.
            # PV uses the exp tile E^T[k, q] as the *stationary* operand
            # (128-column q-chunks) and V[k, e] as the moving operand, so the
            # attention output lands directly in [q, e] — no transposes.
            out_ps = ps_acc.tile([P, n_qt, EMB], F32, tag="po",
                                 name=f"out_ps_{qb_i}")
            # two interleaved softmax-denominator accumulators, one summed on
            # the vector engine and one on gpsimd, so neither chain bottlenecks
            dacc = [work.tile([P, QB], F32, tag=f"dacc{i}", name=f"dacc{i}_{qb_i}")
                    for i in range(2)]
            elist = []
            for kt_i in range(n_kt):
                ksl = slice(kt_i * P, (kt_i + 1) * P)
                stp = ps_mm.tile([P, QB], F32, tag="mm")
                for t in range(2):
                    # lhsT (KT slice) is reused by both q-halves: one
                    # LDWEIGHTS serves two matmuls.
                    for qh in range(2):
                        hs = slice(qh * MMN, (qh + 1) * MMN)
                        nc.tensor.matmul(stp[:, hs], KT[t][:, ksl],
                                         QT[t][:, q0b + qh * MMN:
                                                q0b + (qh + 1) * MMN],
                                         start=(t == 0), stop=(t == 1))
                if kt_i >= 4:
                    # PV lags four k-tiles: extra slack on the exp dependency.
                    kp = kt_i - 4
                    for j in range(n_qt):
                        # PSUM accumulation groups are bank-granular: two
                        # adjacent j-slices share a 2KB bank, so the group
                        # opens on the even slice and closes on the odd one.
                        nc.tensor.matmul(out_ps[:, j, :],
                                         elist[kp][:, j * P:(j + 1) * P],
                                         Vb[:, kp, :],
                                         start=(kp == 0 and j % 2 == 0),
                                         stop=False)
                ebf = work.tile([P, QB], BF16, tag="E", bufs=8)
                nc.scalar.activation(ebf, stp, AF.Exp, scale=scale)
                idx = kt_i % 2
                eng = nc.vector
                if kt_i < 2:
                    eng.tensor_copy(dacc[idx], ebf)
                else:
                    eng.tensor_add(dacc[idx], dacc[idx], ebf)
                elist.append(ebf)
            for kp in (n_kt - 4, n_kt - 3, n_kt - 2, n_kt - 1):
                for j in range(n_qt):
                    nc.tensor.matmul(out_ps[:, j, :],
                                     elist[kp][:, j * P:(j + 1) * P],
                                     Vb[:, kp, :],
                                     start=False, stop=(kp == n_kt - 1 and j % 2 == 1))

            dall = work.tile([P, QB], F32, tag="dall")
            nc.vector.tensor_add(dall, dacc[0], dacc[1])

            # softmax denominators via tiny N=1 matmuls: dn[:, j] =
            # dall_chunk_j.T @ ones -> [q, 1] columns, accumulated in a single
            # PSUM bank (group opens at j=0, closes at j=n_qt-1).
            dn_ps = ps_mm.tile([P, n_qt], F32, tag="mm", name=f"dn_{qb_i}")
            for j in range(n_qt):
                nc.tensor.matmul(dn_ps[:, j:j + 1],
                                 dall[:, j * P:(j + 1) * P], ones_f,
                                 start=(j == 0), stop=(j == n_qt - 1))
            recip = work.tile([P, n_qt], F32, tag="recip")
            nc.vector.reciprocal(recip, dn_ps)

            # finalize: out = out_ps*recip + bv, fused in one DVE op per tile
            for j in range(n_qt):
                res = outp.tile([P, EMB], F32, tag="res")
                nc.vector.scalar_tensor_tensor(
                    res, out_ps[:, j, :], recip[:, j:j + 1], bv_bc,
                    op0=mybir.AluOpType.mult, op1=mybir.AluOpType.add)
                q0 = q0b + j * P
                nc.sync.dma_start(out[q0:q0 + P, :], res)


def _make_nc(s_len: int = S) -> bass.Bass:
    # Bacc (not raw Bass): its compile() splits multi-sem waits and moves
    # matmul waits onto ldweights — HW allows at most one wait per inst.
    nc = bacc.Bacc("TRN2", target_bir_lowering=False, debug=False)
    _build(nc, s_len)
    nc.compile()
    return nc


def _prep(inputs: dict) -> dict:
    arrs = {k: np.ascontiguousarray(np.asarray(v, dtype=np.float32))
            for k, v in inputs.items()}
    assert arrs["x"].shape == (B, S, EMB), arrs["x"].shape
    return arrs


def run(inputs: dict):
    """Run on 8 NeuronCores. Returns (out[B,S,E] f32, BassKernelResults)."""
    arrs = _prep(inputs)
    nc = _make_nc(S)
    shared = {k: arrs[k] for k in ("Wq", "bq", "Wk", "bk", "Wv", "bv")}
    in_maps = [dict(shared, x=arrs["x"][i]) for i in range(B)]
    res = bass_utils.run_bass_kernel_spmd(nc, in_maps, core_ids=list(range(B)))
    out = np.stack([r["out"] for r in res.results], axis=0).astype(np.float32)
    return out, res


def kernel(**inputs) -> np.ndarray:
    out, _ = run(inputs)
    return out


def bench(inputs: dict, iters: int = 5, chain: int = 1):
    """Compile once, then time repeated executions with device-resident
    inputs (mirrors bass2jax.run_bass_via_pjrt's multi-core path).

    `chain` > 1 executes the NEFF that many times inside one XLA program
    (each call's outputs feed the next call's donated output buffers, which
    serializes them) so per-iteration device time can be extracted as a
    slope, amortizing the axon dispatch overhead.

    Returns (out[B,S,E] f32, list of per-call wall times in seconds).
    """
    import time

    import jax
    from jax.sharding import Mesh, NamedSharding, PartitionSpec
    from jax.experimental.shard_map import shard_map

    from concourse import bass2jax
    from concourse import mybir as mb

    arrs = _prep(inputs)
    nc = _make_nc(S)
    bass2jax.install_neuronx_cc_hook()

    partition_name = (
        nc.partition_id_tensor.name if nc.partition_id_tensor else None
    )
    in_names, out_names, out_avals, zero_outs = [], [], [], []
    for alloc in nc.m.functions[0].allocations:
        if not isinstance(alloc, mb.MemoryLocationSet):
            continue
        name = alloc.memorylocations[0].name
        if alloc.kind == "ExternalInput":
            if name != partition_name:
                in_names.append(name)
        elif alloc.kind == "ExternalOutput":
            out_names.append(name)
            shape = tuple(alloc.tensor_shape)
            dtype = mb.dt.np(alloc.dtype)
            out_avals.append(jax.core.ShapedArray(shape, dtype))
            zero_outs.append(np.zeros(shape, dtype))
    n_params = len(in_names)
    n_outs = len(out_avals)
    all_names = in_names + out_names
    if partition_name is not None:
        all_names = all_names + [partition_name]

    def _call(ins, zeros):
        operands = list(ins) + list(zeros)
        if partition_name is not None:
            operands.append(bass2jax.partition_id_tensor())
        return bass2jax._bass_exec_p.bind(
            *operands,
            out_avals=tuple(out_avals),
            in_names=tuple(all_names),
            out_names=tuple(out_names),
            lowering_input_output_aliases=(),
            sim_require_finite=True,
            sim_require_nnan=True,
            nc=nc,
        )

    def _body(*args):
        ins = list(args[:n_params])
        zeros = list(args[n_params:])
        outs = _call(ins, zeros)
        for _ in range(chain - 1):
            outs = _call(ins, list(outs))
        return tuple(outs)

    devices = jax.devices()[:B]
    mesh = Mesh(np.asarray(devices), ("core",))
    in_specs = (PartitionSpec("core"),) * (n_params + n_outs)
    out_specs = (PartitionSpec("core"),) * n_outs
    donate = tuple(range(n_params, n_params + n_outs))
    sharded = jax.jit(
        shard_map(_body, mesh=mesh, in_specs=in_specs, out_specs=out_specs,
                  check_rep=False),
        donate_argnums=donate,
        keep_unused=True,
    )

    per_core = [
        [arrs["x"][c] if n == "x" else arrs[n] for n in in_names[:n_params]]
        for c in range(B)
    ]
    concat_in = [
        np.concatenate([per_core[c][i] for c in range(B)], axis=0)
        for i in range(n_params)
    ]
    concat_zeros = [
        np.zeros((B * z.shape[0], *z.shape[1:]), z.dtype) for z in zero_outs
    ]

    shard = NamedSharding(mesh, PartitionSpec("core"))
    dev_in = [jax.device_put(a, shard) for a in concat_in]
    jax.block_until_ready(dev_in)

    times = []
    out_np = None
    for i in range(iters + 1):
        dev_zeros = [jax.device_put(z, shard) for z in concat_zeros]
        jax.block_until_ready(dev_zeros)
        t0 = time.perf_counter()
        outs = sharded(*dev_in, *dev_zeros)
        jax.block_until_ready(outs)
        dt = time.perf_counter() - t0
        if i == 0:
            idx = out_names.index("out")
            out_np = np.asarray(outs[idx]).reshape(B, S, EMB).astype(np.float32)
        else:
            times.append(dt)
    return out_np, times
